# revision 1
# baseline (speedup 1.0000x reference)
"""Minibatch discrimination kernel for 8 trn2 NeuronCores.

reference:
    M = (x @ T).reshape(B, K, D)                       # B=1024, K=50, D=5
    abs_diffs[i,k,j] = sum_d |M[i,k,d] - M[j,k,d]|
    feat[i,k] = sum_j exp(-abs_diffs[i,k,j])
    out = concat([x, feat], axis=1)                    # [1024, 562]

Sharding: rows of x (batch) split across 8 cores, 128 query rows each.
Every core recomputes the full M^T (cheap) so no collectives are needed.

Per core mapping (i = 128 local query rows on partitions, j = 1024 keys on
the free axis):
 - PE broadcasts row c of M^T across 128 partitions with a one-hot matmul
   (one-hot lhsT stationary, 32-row-aligned slice of M^T moving).
 - ScalarE computes |M_i - M_j| = Abs(-psum + bias) with per-partition bias
   M_local[:, c] for 4 of 5 planes; DVE covers the 5th via
   |d| = relu(d) - min(d, 0) (two tensor_scalar ops + a subtract).
 - fp16 tensor_tensor adds accumulate the 5 planes into L1.
 - ScalarE Exp(-L1) with accum_out produces feat[:, k] (row sum fused).
"""

import sys

sys.path.insert(0, "/opt/trn_rl_repo")

from contextlib import ExitStack

import numpy as np

import concourse.bass as bass
import concourse.bacc as bacc
import concourse.tile as tile
from concourse import mybir
from concourse.bass_utils import run_bass_kernel_spmd

B, F = 1024, 512
K, D = 50, 5
C = K * D  # 250 columns of M
NCORES = 8
ROWS = B // NCORES  # 128 query rows per core

f32 = mybir.dt.float32
f16 = mybir.dt.float16

# planes the scalar engine drains; the last plane goes to DVE (relu-min pair)
SCALAR_PLANES = (0, 1, 2, 3)


def _build_program():
    nc = bacc.Bacc("TRN2", target_bir_lowering=False)

    xT = nc.dram_tensor("xT", [F, B], f32, kind="ExternalInput").ap()
    xTloc = nc.dram_tensor("xTloc", [F, ROWS], f32, kind="ExternalInput").ap()
    Tm = nc.dram_tensor("Tm", [F, C], f32, kind="ExternalInput").ap()
    onehot = nc.dram_tensor("onehot", [128, 32 * 128], f16, kind="ExternalInput").ap()
    feat = nc.dram_tensor("feat", [ROWS, K], f32, kind="ExternalOutput").ap()

    with tile.TileContext(nc) as tc, ExitStack() as ctx:
        const_pool = ctx.enter_context(tc.tile_pool(name="const", bufs=1))
        build_psum = ctx.enter_context(tc.tile_pool(name="bpsum", bufs=1, space="PSUM"))
        bc_psum = ctx.enter_context(tc.tile_pool(name="bcpsum", bufs=3, space="PSUM"))
        plane_pool = ctx.enter_context(tc.tile_pool(name="planes", bufs=12))
        tmp_pool = ctx.enter_context(tc.tile_pool(name="tmps", bufs=6))
        scratch_pool = ctx.enter_context(tc.tile_pool(name="scratch", bufs=4))

        # ---- load inputs -------------------------------------------------
        xt_sb = []
        t_sb = []
        xtl_sb = []
        for fc in range(4):
            t = const_pool.tile([128, B], f32, tag=f"xt{fc}")
            nc.sync.dma_start(out=t[:], in_=xT[128 * fc : 128 * (fc + 1), :])
            xt_sb.append(t)
            t2 = const_pool.tile([128, C], f32, tag=f"tm{fc}")
            nc.sync.dma_start(out=t2[:], in_=Tm[128 * fc : 128 * (fc + 1), :])
            t_sb.append(t2)
            t3 = const_pool.tile([128, ROWS], f32, tag=f"xtl{fc}")
            nc.sync.dma_start(out=t3[:], in_=xTloc[128 * fc : 128 * (fc + 1), :])
            xtl_sb.append(t3)
        oh_sb = const_pool.tile([128, 32 * 128], f16, tag="onehot")
        nc.sync.dma_start(out=oh_sb[:], in_=onehot[:, :])

        # PE may carry at most one sync wait per fused matmul (walrus
        # S3_LW limit). Give PE one dummy matmul per DMA-queue sem it will
        # need, so every real matmul below waits on at most one new sem.
        ps_dummy = build_psum.tile([128, 512], f32, tag="bld", name="ps_dummy")
        for dt_tile in (xt_sb[0], xt_sb[1], xt_sb[2], xt_sb[3], oh_sb):
            nc.tensor.matmul(
                out=ps_dummy[:, :],
                lhsT=dt_tile[0:32, 0:128],
                rhs=dt_tile[0:32, 0:512],
                start=True,
                stop=True,
                tile_position=(0, 0),
            )

        # ---- build M^T ([250,1024] as 2 tiles of [128,1024]) -------------
        mt_sb = [
            const_pool.tile([128, B], f16, tag="mt0", name="mt0"),
            const_pool.tile([128, B], f16, tag="mt1", name="mt1"),
        ]
        # zero block 1 first so its 6 pad rows never feed junk into the matmul
        nc.vector.memset(mt_sb[1][:, :], 0.0)
        for blk in range(2):
            cw = 128 if blk == 0 else C - 128  # 128, then 122
            for jh in range(2):
                ps = build_psum.tile([128, 512], f32, tag="bld")
                for fc in range(4):
                    nc.tensor.matmul(
                        out=ps[:cw, :],
                        lhsT=t_sb[fc][:, 128 * blk : 128 * blk + cw],
                        rhs=xt_sb[fc][:, 512 * jh : 512 * (jh + 1)],
                        start=(fc == 0),
                        stop=(fc == 3),
                    )
                nc.scalar.copy(mt_sb[blk][:cw, 512 * jh : 512 * (jh + 1)], ps[:cw, :])

        # ---- build M_local [128, 250] ------------------------------------
        mloc = const_pool.tile([128, C], f32, tag="mloc")
        ps = build_psum.tile([128, 512], f32, tag="bld")
        for fc in range(4):
            nc.tensor.matmul(
                out=ps[:, :C],
                lhsT=xtl_sb[fc][:],
                rhs=t_sb[fc][:],
                start=(fc == 0),
                stop=(fc == 3),
            )
        nc.scalar.copy(mloc[:], ps[:, :C])

        feat_sb = const_pool.tile([128, K], f32, tag="feat")
        zeros16 = const_pool.tile([128, B], f16, tag="zeros16")
        nc.vector.memset(zeros16[:, :], 0.0)

        # ---- main loop over the 50 kernels -------------------------------
        for k in range(K):
            planes = []
            for d in range(D):
                c = 5 * k + d
                blk, r = divmod(c, 128)
                bbase = (r // 32) * 32
                c0 = r % 32
                ps = bc_psum.tile([128, B], f32, tag="bc")
                for jh in range(2):
                    nc.tensor.matmul(
                        out=ps[:, 512 * jh : 512 * (jh + 1)],
                        lhsT=oh_sb[bbase : bbase + 32, 128 * c0 : 128 * (c0 + 1)],
                        rhs=mt_sb[blk][bbase : bbase + 32, 512 * jh : 512 * (jh + 1)],
                        start=True,
                        stop=True,
                        tile_position=(bbase, 0),
                    )
                pl = plane_pool.tile([128, B], f16, tag="plane")
                if d in SCALAR_PLANES:
                    nc.scalar.activation(
                        pl[:],
                        ps[:],
                        mybir.ActivationFunctionType.Abs,
                        bias=mloc[:, c : c + 1],
                        scale=-1.0,
                    )
                else:
                    # |diff| = relu(diff) - min(diff, 0), all walrus-legal ops
                    pa = plane_pool.tile([128, B], f16, tag="pa")
                    nc.vector.tensor_scalar(
                        pa[:], ps[:], mloc[:, c : c + 1], 0.0,
                        op0=mybir.AluOpType.subtract, op1=mybir.AluOpType.max,
                    )
                    pb = plane_pool.tile([128, B], f16, tag="pb")
                    nc.vector.tensor_scalar(
                        pb[:], ps[:], mloc[:, c : c + 1], 0.0,
                        op0=mybir.AluOpType.subtract, op1=mybir.AluOpType.min,
                    )
                    nc.vector.tensor_tensor(
                        out=pl[:], in0=pa[:], in1=pb[:], op=mybir.AluOpType.subtract
                    )
                planes.append(pl)

            t01 = tmp_pool.tile([128, B], f16, tag="t01")
            nc.vector.tensor_tensor(
                out=t01[:], in0=planes[0][:], in1=planes[1][:], op=mybir.AluOpType.add
            )
            t23 = tmp_pool.tile([128, B], f16, tag="t23")
            nc.vector.tensor_tensor(
                out=t23[:], in0=planes[2][:], in1=planes[3][:], op=mybir.AluOpType.add
            )
            t0123 = tmp_pool.tile([128, B], f16, tag="t0123")
            nc.vector.tensor_tensor(
                out=t0123[:], in0=t01[:], in1=t23[:], op=mybir.AluOpType.add
            )
            l1 = tmp_pool.tile([128, B], f16, tag="l1")
            nc.vector.tensor_tensor(
                out=l1[:], in0=t0123[:], in1=planes[4][:], op=mybir.AluOpType.add
            )

            ex = scratch_pool.tile([128, B], f16, tag="ex")
            nc.scalar.activation(
                ex[:],
                l1[:],
                mybir.ActivationFunctionType.Exp,
                bias=0.0,
                scale=-1.0,
                accum_out=feat_sb[:, k : k + 1],
            )

        nc.sync.dma_start(out=feat[:, :], in_=feat_sb[:, :K])

    nc.compile()
    return nc


_program_cache = {}


def _get_program():
    if "nc" not in _program_cache:
        _program_cache["nc"] = _build_program()
    return _program_cache["nc"]


def _make_onehot():
    oh = np.zeros((128, 32 * 128), dtype=np.float16)
    for p in range(128):
        oh[p, (p % 32) * 128 : (p % 32 + 1) * 128] = 1.0
    return oh


def kernel(x: np.ndarray, T: np.ndarray, _trace=False, _trace_kwargs=None):
    x = np.asarray(x, dtype=np.float32)
    T = np.asarray(T, dtype=np.float32)
    nc = _get_program()

    xT_full = np.ascontiguousarray(x.T)  # [512, 1024]
    oh = _make_onehot()
    in_maps = []
    for i in range(NCORES):
        in_maps.append(
            {
                "xT": xT_full,
                "xTloc": np.ascontiguousarray(x.T[:, ROWS * i : ROWS * (i + 1)]),
                "Tm": T,
                "onehot": oh,
            }
        )

    res = run_bass_kernel_spmd(
        nc,
        in_maps,
        core_ids=list(range(NCORES)),
        trace=_trace,
        **(_trace_kwargs or {}),
    )
    feats = np.concatenate([res.results[i]["feat"] for i in range(NCORES)], axis=0)
    out = np.concatenate([x, feats.astype(np.float32)], axis=1)
    if _trace:
        return out, res
    return out



# revision 3
# speedup vs baseline: 2.1125x; 2.1125x over previous
"""Minibatch discrimination kernel for 8 trn2 NeuronCores — v3.

reference:
    M = (x @ T).reshape(B, K, D)                       # B=1024, K=50, D=5
    abs_diffs[i,k,j] = sum_d |M[i,k,d] - M[j,k,d]|
    feat[i,k] = sum_j exp(-abs_diffs[i,k,j])
    out = concat([x, feat], axis=1)                    # [1024, 562]

Sharding: kernels k split across 8 cores (K padded 50->56, 7 per core);
each core computes feat[:, its 7 k's] for ALL 1024 rows. The j-broadcast
of M^T rows is amortized over the 8 query i-tiles and done by DMA from a
DRAM staging buffer (SBUF sources would need partition alignment).

abs trick (|x| ops are not ISA-valid on DVE): |d| = d + 2*relu(-d), so
    L1[i,j] = (S[j] - S[i]) - 2*sum_d min(M[j,d]-M[i,d], 0),
    S[j] = sum_d M[j,d].
Per (k, i-tile) unit, exploiting symmetry of E = exp(-L1):
 - only j >= 128*it is computed (upper block-triangle, ~56% of work)
 - DVE: plane_d = min(bcast_d - mloc[:,d], 0) via ONE fused tensor_scalar
   (subtract -> min with 0), fp16 SBUF in/out, 4x perf mode
 - PE: matmuls accumulate into PSUM: S-row broadcast via +I, the 5 min
   planes via the stationary -2*I (scale and subtraction ride the
   weights); for wide tiles DVE/Pool pre-add plane pairs to offload PE
 - Scalar: E = Exp(-PSUM + S_local) via activation bias, fp16 out,
   accum_out = row-sum (diagonal + right-of-diagonal feat contribution)
 - PE: ones-vector matmuls column-sum E's off-diagonal 128-blocks =
   mirrored contribution to later i-tiles' feat
 - DVE: feat[:, (it,k)] = diag accum + mirrored accums
The S[j] term rides the broadcast as a 6th row per kernel slot; S_local
rides the M_local matmul as a 6th column (host passes [T | colsum(T)]
interleaved per slot) and enters exp via the bias operand.
"""

import sys

sys.path.insert(0, "/opt/trn_rl_repo")

from contextlib import ExitStack

import numpy as np

import concourse.bass as bass
import concourse.bacc as bacc
import concourse.tile as tile
from concourse import mybir
from concourse.bass_utils import run_bass_kernel_spmd

B, F = 1024, 512
K, D = 50, 5
NCORES = 8
KC = 7                # kernels per core (K padded to 56)
SW = D + 1            # 6 staged rows/cols per kernel slot (5 m + 1 S)
CC = KC * SW          # 42 staged M^T rows / M_local cols per core
NT = 8                # query i-tiles of 128 rows

f32 = mybir.dt.float32
f16 = mybir.dt.float16

# per-i-tile plane pre-add engine: planes 3+4 summed by 'pool'/'dve'
# before PE streams the remaining summands into PSUM; None = PE does all
PRE = ["pool", "pool", "pool", "pool", "dve", "dve", None, None]

SUB = mybir.AluOpType.subtract
MIN = mybir.AluOpType.min
ADD = mybir.AluOpType.add


def _build_program():
    nc = bacc.Bacc("TRN2", target_bir_lowering=False)

    xT = nc.dram_tensor("xT", [F, B], f16, kind="ExternalInput").ap()
    Tloc = nc.dram_tensor("Tloc", [F, CC], f16, kind="ExternalInput").ap()
    nident2 = nc.dram_tensor("nident2", [128, 128], f16, kind="ExternalInput").ap()
    ones = nc.dram_tensor("ones", [128, 1], f16, kind="ExternalInput").ap()
    feat = nc.dram_tensor("feat", [B, KC], f32, kind="ExternalOutput").ap()
    # DRAM staging for broadcast rows: DMA-broadcast sources must be
    # partition-aligned in SBUF, but DRAM APs carry no such constraint.
    mt_dram = nc.dram_tensor("mt_dram", [CC, B], f16, kind="Internal").ap()

    with tile.TileContext(nc) as tc, ExitStack() as ctx:
        const_pool = ctx.enter_context(tc.tile_pool(name="const", bufs=1))
        mm_psum = ctx.enter_context(tc.tile_pool(name="mmps", bufs=1, space="PSUM"))
        l1_psum = ctx.enter_context(tc.tile_pool(name="l1ps", bufs=2, space="PSUM"))
        cs_psum = ctx.enter_context(tc.tile_pool(name="csps", bufs=2, space="PSUM"))
        bc_pool = ctx.enter_context(tc.tile_pool(name="bc", bufs=3))
        plane_pool = ctx.enter_context(tc.tile_pool(name="planes", bufs=3))
        e_pool = ctx.enter_context(tc.tile_pool(name="etile", bufs=3))
        small_pool = ctx.enter_context(tc.tile_pool(name="small", bufs=4))

        # ---- load inputs -------------------------------------------------
        xt_sb = []
        tl_sb = []
        for fc in range(4):
            t = const_pool.tile([128, B], f16, tag=f"xt{fc}")
            nc.sync.dma_start(out=t[:], in_=xT[128 * fc : 128 * (fc + 1), :])
            xt_sb.append(t)
            t2 = const_pool.tile([128, CC], f16, tag=f"tl{fc}")
            nc.sync.dma_start(out=t2[:], in_=Tloc[128 * fc : 128 * (fc + 1), :])
            tl_sb.append(t2)
        n2_sb = const_pool.tile([128, 128], f16, tag="nident2")
        nc.sync.dma_start(out=n2_sb[:], in_=nident2[:, :])
        ones_sb = const_pool.tile([128, 1], f16, tag="ones")
        nc.sync.dma_start(out=ones_sb[:], in_=ones[:, :])

        # PE may carry at most one sync wait per fused matmul (walrus
        # S3_LW limit); give PE a dummy matmul per input-DMA sem so real
        # matmuls below wait on at most one new sem.
        ps_dummy = mm_psum.tile([128, 512], f32, tag="mm", name="ps_dummy")
        for dt_tile in xt_sb + tl_sb + [n2_sb]:
            dw = min(dt_tile.shape[1], 128)
            nc.tensor.matmul(
                out=ps_dummy[0:dw, 0:dw],
                lhsT=dt_tile[0:32, 0:dw],
                rhs=dt_tile[0:32, 0:dw],
                start=True,
                stop=True,
                tile_position=(0, 0),
            )

        # ---- staged rows [CC, 1024] fp16: per slot 5 m-rows + 1 S-row ----
        # fast path: slot 0's rows first, so the k=0 broadcast (and the
        # whole DVE pipeline) starts ~10us earlier
        mt0_sb = const_pool.tile([128, B], f16, tag="mt0")
        for jh in range(2):
            ps = mm_psum.tile([128, 512], f32, tag="mm")
            for fc in range(4):
                nc.tensor.matmul(
                    out=ps[:SW, :],
                    lhsT=tl_sb[fc][:, :SW],
                    rhs=xt_sb[fc][:, 512 * jh : 512 * (jh + 1)],
                    start=(fc == 0),
                    stop=(fc == 3),
                )
            nc.scalar.copy(mt0_sb[:SW, 512 * jh : 512 * (jh + 1)], ps[:SW, :])
        nc.sync.dma_start(out=mt_dram[:SW, :], in_=mt0_sb[:SW, :])

        mt_sb = const_pool.tile([128, B], f16, tag="mt")
        for jh in range(2):
            ps = mm_psum.tile([128, 512], f32, tag="mm")
            for fc in range(4):
                nc.tensor.matmul(
                    out=ps[: CC - SW, :],
                    lhsT=tl_sb[fc][:, SW:CC],
                    rhs=xt_sb[fc][:, 512 * jh : 512 * (jh + 1)],
                    start=(fc == 0),
                    stop=(fc == 3),
                )
            nc.scalar.copy(
                mt_sb[: CC - SW, 512 * jh : 512 * (jh + 1)], ps[: CC - SW, :]
            )
        nc.sync.dma_start(out=mt_dram[SW:, :], in_=mt_sb[: CC - SW, :])

        # ---- M_local [128, (it, c)] f32 (c = slot-major, 6 per slot) -----
        mloc = const_pool.tile([128, NT * CC], f32, tag="mloc")
        for it in range(NT):
            ps = mm_psum.tile([128, 512], f32, tag="mm")
            for fc in range(4):
                nc.tensor.matmul(
                    out=ps[:, :CC],
                    lhsT=xt_sb[fc][:, 128 * it : 128 * (it + 1)],
                    rhs=tl_sb[fc][:, :CC],
                    start=(fc == 0),
                    stop=(fc == 3),
                )
            nc.scalar.copy(mloc[:, it * CC : (it + 1) * CC], ps[:, :CC])

        # exp bias: S_local = -2 * (the staged -S/2 columns of mloc)
        sbias = const_pool.tile([128, NT * KC], f32, tag="sbias")
        mls = mloc[:].rearrange("p (t k s) -> p t k s", k=KC, s=SW)
        nc.vector.tensor_scalar(
            sbias[:, :],
            mls[:, :, :, D : D + 1],
            -2.0,
            None,
            op0=mybir.AluOpType.mult,
        )

        feat_sb = const_pool.tile([128, NT * KC], f32, tag="feat")
        fv = feat_sb[:].rearrange("p (t k) -> p t k", t=NT)

        # ---- main loop over this core's KC kernels -----------------------
        for k in range(KC):
            # broadcast slot k's 6 staged rows to all partitions, one DMA
            # per row so consumers start as soon as their row lands
            bc = bc_pool.tile([128, SW * B], f16, tag="bc")
            for d in range(SW):
                r = SW * k + d
                src = mt_dram[r, :].partition_broadcast(128)
                eng = nc.gpsimd if d % 2 == 0 else nc.sync
                eng.dma_start(out=bc[:, d * B : (d + 1) * B], in_=src)

            diag = small_pool.tile([128, NT], f32, tag="diag")
            # mirrored-contribution accumulators; two to halve the serial
            # add chain (even its -> mirA, odd -> mirB)
            mirA = small_pool.tile([128, NT], f32, tag="mirA")
            mirB = small_pool.tile([128, NT], f32, tag="mirB")
            nc.gpsimd.memset(mirA[:, :], 0.0)
            nc.gpsimd.memset(mirB[:, :], 0.0)

            for it in range(NT):
                off = 128 * it
                w = B - off
                mc = it * CC + SW * k
                planes = plane_pool.tile([128, D * B], f16, tag="pl")
                for d in range(D):
                    nc.vector.tensor_scalar(
                        planes[:, d * w : (d + 1) * w],
                        bc[:, d * B + off : (d + 1) * B],
                        mloc[:, mc + d : mc + d + 1],
                        0.0,
                        op0=SUB,
                        op1=MIN,
                    )
                # all summands stream through the stationary -2*I: the 5 min
                # planes directly; the staged row holds -S/2 so it lands as
                # +S[j]. Where a pre-add runs, it folds -S/2 into plane 3.
                sv = bc[:, D * B + off : (D + 1) * B]
                streams = [planes[:, d * w : (d + 1) * w] for d in range(D)]
                if PRE[it] is not None:
                    p34 = small_pool.tile([128, B], f16, tag="p34")
                    eng = nc.vector if PRE[it] == "dve" else nc.gpsimd
                    eng.tensor_tensor(
                        out=p34[:, :w], in0=streams[3], in1=sv, op=ADD
                    )
                    streams = streams[:3] + [streams[4], p34[:, :w]]
                else:
                    streams = streams + [sv]

                l1 = l1_psum.tile([128, B], f32, tag="l1")
                for c0 in range(0, w, 512):
                    c1 = min(c0 + 512, w)
                    for si, srcp in enumerate(streams):
                        nc.tensor.matmul(
                            out=l1[:, c0:c1],
                            lhsT=n2_sb[:, :],
                            rhs=srcp[:, c0:c1],
                            start=(si == 0),
                            stop=(si == len(streams) - 1),
                        )

                e = e_pool.tile([128, B], f16, tag="e")
                nc.scalar.activation(
                    e[:, :w],
                    l1[:, :w],
                    mybir.ActivationFunctionType.Exp,
                    bias=sbias[:, it * KC + k : it * KC + k + 1],
                    scale=-1.0,
                    accum_out=diag[:, it : it + 1],
                )

                # column-sums of off-diagonal 128-blocks -> mirrored feat
                if it < NT - 1:
                    cs = cs_psum.tile([128, NT], f32, tag="cs")
                    for jt in range(it + 1, NT):
                        lo = 128 * (jt - it)
                        nc.tensor.matmul(
                            out=cs[:, jt : jt + 1],
                            lhsT=e[:, lo : lo + 128],
                            rhs=ones_sb[:, :],
                            start=True,
                            stop=True,
                        )
                    mir = mirA if it % 2 == 0 else mirB
                    nc.vector.tensor_tensor(
                        out=mir[:, it + 1 : NT],
                        in0=mir[:, it + 1 : NT],
                        in1=cs[:, it + 1 : NT],
                        op=ADD,
                    )

            dm = small_pool.tile([128, NT], f32, tag="dm")
            nc.vector.tensor_tensor(
                out=dm[:, :], in0=mirA[:, :], in1=diag[:, 0:NT], op=ADD
            )
            nc.vector.tensor_tensor(
                out=fv[:, 0:NT, k : k + 1],
                in0=dm[:, :],
                in1=mirB[:, :],
                op=ADD,
            )

        out_view = feat[:, :].rearrange("(t p) k -> p t k", t=NT)
        nc.sync.dma_start(out=out_view, in_=fv)

    nc.compile()
    return nc


_program_cache = {}


def _get_program():
    if "nc" not in _program_cache:
        _program_cache["nc"] = _build_program()
    return _program_cache["nc"]


def _make_consts():
    nident2 = np.zeros((128, 128), dtype=np.float16)
    np.fill_diagonal(nident2, -2.0)
    ones = np.ones((128, 1), dtype=np.float16)
    return nident2, ones


def make_in_maps(x, T):
    xT_full = np.ascontiguousarray(x.T.astype(np.float16))  # [512, 1024]
    T_pad = np.zeros((F, NCORES * KC * D), dtype=np.float32)
    T_pad[:, : K * D] = T
    nident2, ones = _make_consts()
    in_maps = []
    for i in range(NCORES):
        Ts = T_pad[:, KC * D * i : KC * D * (i + 1)].reshape(F, KC, D)
        Tl = np.zeros((F, KC, SW), dtype=np.float32)
        Tl[:, :, :D] = Ts
        Tl[:, :, D] = -0.5 * Ts.sum(axis=2)
        in_maps.append(
            {
                "xT": xT_full,
                "Tloc": np.ascontiguousarray(
                    Tl.reshape(F, CC).astype(np.float16)
                ),
                "nident2": nident2,
                "ones": ones,
            }
        )
    return in_maps


def kernel(x: np.ndarray, T: np.ndarray, _trace=False, _trace_kwargs=None):
    x = np.asarray(x, dtype=np.float32)
    T = np.asarray(T, dtype=np.float32)
    nc = _get_program()
    in_maps = make_in_maps(x, T)

    res = run_bass_kernel_spmd(
        nc,
        in_maps,
        core_ids=list(range(NCORES)),
        trace=_trace,
        **(_trace_kwargs or {}),
    )
    feats = np.concatenate(
        [res.results[i]["feat"] for i in range(NCORES)], axis=1
    )[:, :K]
    out = np.concatenate([x, feats.astype(np.float32)], axis=1)
    if _trace:
        return out, res
    return out
